# revision 16
# baseline (speedup 1.0000x reference)
"""Block-diagonal linear (DiagonalLinear) Trainium2 kernel.

y[:, n*256:(n+1)*256] = x[:, n*256:(n+1)*256] @ W[n].T + b[n]  for n in 0..63

Sharding: expert-parallel over the 64 blocks — core c owns blocks
[8c, 8c+8). Host pre-transposes x and W so the contraction dim (ip) lands
on SBUF partitions; x is cast to fp8e3 (e3m4 — x is N(0,1), quantization
lands at ~1e-2 scale-rel, inside the 2e-2 gate), W stays fp16 (mixed
fp16 x fp8 matmul is legal; only fp32 operands must match), y fp16.

Why these numbers: the per-core SBUF<->HBM fabric tops out at ~430 GB/s
shared across reads+writes, so fp16-everything (34.7 MB/core) is
DMA-floored at ~97 us. Dropping x to fp8 (26.2 MB/core) moves the floor
below the tensor engine's streaming limit (256 MMs x 512 cols @ 2.4 GHz
= 55.3 us), which then paces the kernel. DoubleRow (0.5 cyc/row) needs
fp8e4/e5 and e4m3's x-quantization error (2.02e-2) already breaches the
gate, so 1 cyc/row fp8e3 is the fastest legal matmul.

Structure notes (measured, don't regress):
 - Loads on nc.sync's HWDGE ring, y stores on gpsimd's SWDGE ring, bias
   on nc.scalar's ring. Stores must never share a ring with loads
   (head-of-line blocking); bias on the sync ring stalls evictions.
 - W is loaded per-block (8 x 128 KiB tiles, one writer each) so the
   first matmul only waits for W[block0]+x[block0] (~1.5 us of DMA), not
   the whole 1 MiB W image.
 - 8 garbage-fp8 warmup matmuls run during the load head: the PE's HAM
   clock gate defaults to 1.2 GHz and needs ~3.4 us of sustained matmul
   activity to unthrottle to 2.4 GHz; warming on garbage before x lands
   saves the ~1.8 us ramp penalty on real work.
 - Block 0 runs kc-outer (first MM gates on one 512 KiB x chunk);
   blocks 1-7 run j-outer (each PSUM accumulation closes after 2 MMs,
   so evictions trail the MM stream by ~0.4 us instead of bunching
   16-deep at the block end — shrinks the post-MM drain tail).
 - y tiles are split into disjoint [128, 2048] halves with their own
   store DMAs: earlier stores, and no writer-after-reader hazard on a
   shared ytile that Tile would serialize.
 - PSUM evictions alternate DVE (tensor_scalar_add) / ACT (activation
   bias) — one engine alone (~730 ns per [128,512] chunk) cannot keep
   up with y production.
"""

from contextlib import ExitStack

import ml_dtypes
import numpy as np

import concourse.bacc as bacc
import concourse.bass as bass
import concourse.tile as tile
from concourse import mybir
from concourse.bass_utils import run_bass_kernel_spmd

N_COPIES, IP, OP, BATCH = 64, 256, 256, 4096
N_CORES = 8
BPC = N_COPIES // N_CORES  # blocks per core
P = 128
KC = IP // P  # contraction chunks per block
MC = OP // P  # output-partition chunks per block
FREE = 512  # moving free dim per matmul (one PSUM bank of fp32)
JN = BATCH // FREE
HB = BATCH // 2  # half-batch store granularity

_prog_cache = {}


def _build_program():
    nc = bacc.Bacc("TRN2", target_bir_lowering=False, debug=False)
    f32 = mybir.dt.float32
    f16 = mybir.dt.float16
    f8 = mybir.dt.float8e3

    xt = nc.dram_tensor("xt", [BPC, IP, BATCH], f8, kind="ExternalInput").ap()
    # wt/bb arrive pre-packed partition-major: wt[p, n*KC+kc, o], bb[p, n*MC+m]
    wt = nc.dram_tensor("wt", [P, BPC * KC, OP], f16, kind="ExternalInput").ap()
    bb = nc.dram_tensor("bb", [P, BPC * MC], f32, kind="ExternalInput").ap()
    yt = nc.dram_tensor("yt", [BPC, OP, BATCH], f16, kind="ExternalOutput").ap()

    with tile.TileContext(nc) as tc, ExitStack() as ctx:
        const = ctx.enter_context(tc.tile_pool(name="const", bufs=1))
        # all 8 x blocks stay resident (64 KiB/partition) — no recycling,
        # so loads stream at full ring rate with no compute gating.
        xpool = ctx.enter_context(tc.tile_pool(name="x", bufs=1))
        ypool = ctx.enter_context(tc.tile_pool(name="y", bufs=10))
        psum = ctx.enter_context(tc.tile_pool(name="ps", bufs=8, space="PSUM"))

        # PE warmup fodder: zeroed fp8 operands, never read back.
        wdum = const.tile([P, P], f8)
        xdum = const.tile([P, FREE], f8)
        nc.vector.memset(wdum[:], 0.0)
        nc.vector.memset(xdum[:], 0.0)

        # W in exactly two DMAs: block 0 alone (128 KiB — first-MM gate),
        # then blocks 1-7 in one 0.9 MiB transfer. Per-block 128 KiB DMAs
        # measured ~187 GB/s on the ring (issue/descriptor-gen bound) and
        # starved the x stream behind them.
        w0tile = const.tile([P, KC, OP], f16)
        wrest = const.tile([P, (BPC - 1) * KC, OP], f16)
        btile = const.tile([P, BPC * MC], f32)

        # x streams alone on the sync ring; W0/bias/wrest ride the scalar
        # ring in parallel (first MM gates on W0+x0c0 across two rings).
        xtiles = []
        for n in range(BPC):
            xtiles.append(xpool.tile([P, KC, BATCH], f8, name=f"x{n}"))
        for n in range(BPC):
            for kc in range(KC):
                nc.sync.dma_start(
                    out=xtiles[n][:, kc], in_=xt[n, bass.ts(kc, P)]
                )
        nc.scalar.dma_start(out=w0tile[:], in_=wt[:, 0:KC])
        nc.scalar.dma_start(out=btile[:], in_=bb[:])
        nc.scalar.dma_start(out=wrest[:], in_=wt[:, KC : BPC * KC])

        def wslice(n, kc, m):
            if n == 0:
                return w0tile[:, kc, bass.ts(m, P)]
            return wrest[:, (n - 1) * KC + kc, bass.ts(m, P)]

        # Warmup: 8 cold matmuls ~= the 3.4 us HAM window; by the time x
        # lands the PE runs at 2.4 GHz. They deposit into the 8 PSUM
        # banks the first real group then overwrites (start=True).
        warm = [psum.tile([P, FREE], f32, name="warm", tag="ps") for _ in range(8)]
        for i in range(8):
            nc.tensor.matmul(warm[i][:], wdum[:], xdum[:], start=True, stop=True)

        def evict(ytile, yslice, ps, bias, j):
            # split PSUM evictions across DVE and ACT
            if j % 2 == 0:
                nc.vector.tensor_scalar_add(ytile[:, yslice], ps[:], bias)
            else:
                nc.scalar.activation(
                    ytile[:, yslice],
                    ps[:],
                    mybir.ActivationFunctionType.Identity,
                    bias=bias,
                )

        for n in range(BPC):
            xtile = xtiles[n]
            # ZERO stores on gpsimd/SWDGE: its Q7 descriptor emission
            # handicaps SDMA engine 15 by ~15-20% for the whole run, and the
            # kernel end was pinned by that engine's backlog (equal byte
            # split across engines, one engine slow => +10 us tail).
            # Blocks 0-3 store via the scalar HWDGE ring (bias/W are done
            # with it by ~12 us); blocks 4-7 via the sync ring — their
            # dma_starts sit behind all x loads in ring-FIFO order, which
            # is harmless since loads finish before block-4 y is ready.
            store_eng = nc.scalar if n < 4 else nc.sync
            for m in range(MC):
                bias = btile[:, n * MC + m : n * MC + m + 1]
                ytile = ypool.tile([P, BATCH], f16, name="yh")
                if n == 0 and m == 0:
                    # kc-outer: first MM gates only on x chunk kc=0.
                    pss = [psum.tile([P, FREE], f32, name="psj", tag="ps") for _ in range(JN)]
                    for kc in range(KC):
                        for j in range(JN):
                            nc.tensor.matmul(
                                pss[j][:],
                                wslice(n, kc, m),
                                xtile[:, kc, bass.ts(j, FREE)],
                                start=(kc == 0),
                                stop=(kc == KC - 1),
                            )
                    for j in range(JN):
                        evict(ytile, bass.ts(j, FREE), pss[j], bias, j)
                else:
                    # j-outer: each psum closes after KC MMs; evictions
                    # trail the MM stream instead of bunching at the end.
                    for j in range(JN):
                        ps = psum.tile([P, FREE], f32, name="psj", tag="ps")
                        for kc in range(KC):
                            nc.tensor.matmul(
                                ps[:],
                                wslice(n, kc, m),
                                xtile[:, kc, bass.ts(j, FREE)],
                                start=(kc == 0),
                                stop=(kc == KC - 1),
                            )
                        evict(ytile, bass.ts(j, FREE), ps, bias, j)
                store_eng.dma_start(out=yt[n, bass.ts(m, P)], in_=ytile[:])

    nc.compile()
    return nc


def _get_program():
    if "nc" not in _prog_cache:
        _prog_cache["nc"] = _build_program()
    return _prog_cache["nc"]


def _prep_inputs(x, W, b):
    x = np.ascontiguousarray(x, dtype=np.float32)
    W = np.ascontiguousarray(W, dtype=np.float32)
    b = np.ascontiguousarray(b, dtype=np.float32)

    # [B, n*ip] -> [n, ip, B]; two-step transpose is much faster than a
    # direct (1, 2, 0) permute copy (cache-friendly inner strides).
    xa = x.reshape(BATCH, N_COPIES, IP).transpose(1, 0, 2)
    xT = np.ascontiguousarray(xa.transpose(0, 2, 1)).astype(
        ml_dtypes.float8_e3m4
    )  # [n, ip, B] fp8e3
    wT = W.transpose(0, 2, 1).astype(np.float16)  # [n, ip, op]
    # pack to [P, n*KC+kc, op]: partition p holds W rows ip = kc*P + p
    wP = np.ascontiguousarray(
        wT.reshape(N_COPIES, KC, P, OP).transpose(2, 0, 1, 3)
    )  # [P, n, KC, op]
    bP = np.ascontiguousarray(
        b.reshape(N_COPIES, MC, P).transpose(2, 0, 1)
    )  # [P, n, MC]
    return [
        {
            "xt": xT[c * BPC : (c + 1) * BPC],
            "wt": np.ascontiguousarray(
                wP[:, c * BPC : (c + 1) * BPC]
            ).reshape(P, BPC * KC, OP),
            "bb": np.ascontiguousarray(
                bP[:, c * BPC : (c + 1) * BPC]
            ).reshape(P, BPC * MC),
        }
        for c in range(N_CORES)
    ]


def _run(x, W, b, **spmd_kwargs):
    in_maps = _prep_inputs(x, W, b)
    nc = _get_program()
    res = run_bass_kernel_spmd(nc, in_maps, core_ids=list(range(N_CORES)), **spmd_kwargs)

    yT = np.concatenate([res.results[c]["yt"] for c in range(N_CORES)], axis=0).astype(np.float32)
    # [n, op, B] -> [B, n, op] -> [B, n*op]
    ya = np.ascontiguousarray(yT.transpose(0, 2, 1))  # [n, B, op]
    y = np.ascontiguousarray(ya.transpose(1, 0, 2)).reshape(BATCH, N_COPIES * OP)
    return y, res


def kernel(x, W, b):
    y, _ = _run(x, W, b)
    return y
